# revision 57
# baseline (speedup 1.0000x reference)
"""SNN (soft-nearest-neighbor) contrastive loss on 8 Trainium2 NeuronCores.

Math
----
z = concat(x, y) in R^{8192x128};  d_ij = ||z_i - z_j||.
The row max subtracted in the reference cancels mathematically, so
    S0_i  = sum_{j != i} exp(-d_ij)          (device + host gather)
    dp_i  = d_{i, pair(i)}                   (host, O(N*D))
    loss  = mean_i( -log( exp(-dp_i)/S0_i + tiny ) )   (host, trivial)

Symmetry halving
----------------
d_ij is symmetric; each 128-row block R computes exp tiles for column
blocks R..R+32 only (self + 32 forward, cyclically).  Strip = 4224 cols.
Row sums cover the WHOLE strip (the antipodal block, offset 32, is
computed by both partners for their own rows).  Column sums (PE one-hot
matmul into a single [10,512] PSUM accumulator) cover offsets 1..31 and
are scattered on the host into the mirrored rows.

Device pipeline (one SPMD program, 8 cores, rows sharded 1024/core)
------------------------------------------------------------------
PE: fp8e4 DoubleRow matmuls with 65-row k-tiles compute
      Q = u.u^T - hsq_j      (u split into 2x64 dims; the 65th row of
k-tile 0/1 carries ones x -hsq_hi / ones x -hsq_lo, folding the hsq_j
rank-1 term into the main matmul at ~fp16 precision for free, at
0.5 cyc/row), plus an FD-128 bf16 identity matmul adding -LARGE on
the self diagonal.
ACT: w = Sqrt(-Q + hsq_i) straight from PSUM via the per-partition
bias AP (bf16 out), one instruction per triple-buffered [128,1024]
PSUM tile.  Exp is split: the last strip runs on ACT (exp with fused
accum_out row sums; one sqrt->exp table switch, hoisted initial sqrt
table load); N_DVE strips run on DVE via the Schraudolph bit trick --
their sqrt is pre-scaled by A16^2 so a single 2x-mode tensor_tensor
subtract produces codes = int16(B16 - A16*d) = the bf16 bit pattern
of ~exp(-d)*SCALE_COMP.  Row sums of the bitcast-bf16 codes come from
two 2x-mode fold adds plus one accumulate pass; PE accumulates column
sums of every strip into one [10,512] PSUM tile via one-hot lhsT.
Each core gets column-ROTATED operands so every tile index is a
compile-time constant: one identical program for all 8 cores.
"""

import os
import sys
from contextlib import ExitStack

import numpy as np

_TRN_REPO = os.environ.get("TRN_RL_REPO", "/opt/trn_rl_repo")
if _TRN_REPO not in sys.path:
    sys.path.insert(0, _TRN_REPO)

import ml_dtypes

BF16 = ml_dtypes.bfloat16

B = 4096
D = 128
N = 2 * B            # 8192 rows of z
NCORES = 8
RPC = N // NCORES    # 1024 rows per core
S = RPC // 128       # 8 row-subtiles per core
CT = 512             # matmul moving tile (one PSUM bank = 512 f32)
SL = 4224            # strip length: self block + 32 forward blocks
CW = 4096            # colsum window end (blocks 1..31): [base+128, base+CW)
PT = 1024            # PSUM strip tile columns (2 banks)
UCOLS = 5120         # rotated cols touched: [0, 128*(S-1) + SL) = 5120
NCH = 10             # colsum chunks of 512 covering rotated cols [0, 5120)
LARGE = 7296.0       # diagonal nuke: d2 -> 7296, w -> 85.4:
                     #   ACT path exp(-85.4) ~ 8e-38 (bf16 ~ 0)
                     #   DVE path code = B16 - A16*85.4 ~ +474 (tiny value)
N_DVE = 7            # strips whose exp runs on DVE (Schraudolph)

LN2 = float(np.log(2.0))
A16 = 128.0 / LN2    # bf16 exponent-code slope
B16 = 16256.0        # bf16 exponent-code offset (exact in bf16)
# with B16=16256 the Schraudolph decode averages exp(-w)*SCALE_COMP;
# ACT-strip exps are biased by ln(SCALE_COMP) to match, and the host
# divides all device sums by SCALE_COMP.
SCALE_COMP = 1.0406027025852233  # mean of (1+f)/2^f over f~U[0,1)

PROFILE = False
LAST_RESULT = None

_cache = {}


def _build_program():
    import concourse.tile as tile
    from bass_rust import add_dep_helper
    from concourse import bacc, mybir

    f32 = mybir.dt.float32
    f16 = mybir.dt.float16
    bf16 = mybir.dt.bfloat16
    i16 = mybir.dt.int16
    fp8 = mybir.dt.float8e4
    AF = mybir.ActivationFunctionType
    OP = mybir.AluOpType
    PM = mybir.MatmulPerfMode

    nc = bacc.Bacc()

    u8 = mybir.dt.uint8
    h_u2 = nc.declare_dram_parameter("u2", [65, 2, UCOLS], fp8, isOutput=False)
    h_u2w = nc.declare_dram_parameter("u2w", [65, 2, S * 128], fp8, isOutput=False)
    # packed consts, one DMA: dfix | ident | oneh | hsqp | hsqpa (bytes)
    h_consts = nc.declare_dram_parameter("consts", [128, 776], u8, isOutput=False)
    h_s0 = nc.declare_dram_parameter("s0", [128, S + 2], f32, isOutput=True)
    h_cs = nc.declare_dram_parameter("cs", [NCH, CT], f32, isOutput=True)

    dve_strip = [s < N_DVE for s in range(S)]

    # strip for subtile s covers rotated cols [s*128, s*128 + SL)
    with tile.TileContext(nc) as tc, ExitStack() as ctx:
        const = ctx.enter_context(tc.tile_pool(name="const", bufs=1))
        wpool = ctx.enter_context(tc.tile_pool(name="wbuf", bufs=S))
        dpool = ctx.enter_context(tc.tile_pool(name="dump", bufs=2))
        cpool = ctx.enter_context(tc.tile_pool(name="codes", bufs=2))
        pspool = ctx.enter_context(tc.tile_pool(name="ps", bufs=2, space="PSUM"))
        psbig = ctx.enter_context(tc.tile_pool(name="psb", bufs=1, space="PSUM"))
        cspool = ctx.enter_context(tc.tile_pool(name="cps", bufs=1, space="PSUM"))
        misc = ctx.enter_context(tc.tile_pool(name="misc", bufs=1))

        # DMA priority: first matmul's operands first, then packed consts,
        # then the rest of u2 in chunks
        t_u2w = const.tile([65, 2, S * 128], fp8)
        nc.sync.dma_start(out=t_u2w[:], in_=h_u2w[:])
        t_u2 = const.tile([65, 2, UCOLS], fp8)
        edges = [0, 256, 512, 768, 1024, 1536, 2048, 2560, 3072, 3584,
                 4096, 4608, 5120]
        for a, b in zip(edges[:5], edges[1:6]):
            nc.sync.dma_start(out=t_u2[:, :, a:b], in_=h_u2[:, :, a:b])

        t_consts = const.tile([128, 776], u8)
        nc.sync.dma_start(out=t_consts[:], in_=h_consts[:])
        t_dfix = t_consts[:, 0:256].bitcast(bf16)
        t_ident = t_consts[:, 256:512].bitcast(bf16)
        t_oneh = t_consts[:, 512:712].bitcast(bf16)
        t_hsqp = t_consts[:, 712:744].bitcast(f32)
        t_hsqpa = t_consts[:, 744:776].bitcast(f32)

        for a, b in zip(edges[5:-1], edges[6:]):
            nc.sync.dma_start(out=t_u2[:, :, a:b], in_=h_u2[:, :, a:b])

        t_zero10 = const.tile([128, NCH], bf16)
        nc.vector.memset(t_zero10[:], 0.0)
        t_z512 = const.tile([128, CT], bf16)
        nc.vector.memset(t_z512[:], 0.0)
        t_b16 = const.tile([128, SL], bf16)
        nc.gpsimd.memset(t_b16[:], B16)
        t_ebias = const.tile([128, 1], f32)
        nc.vector.memset(t_ebias[:], float(np.log(SCALE_COMP)))

        s0_t = const.tile([128, S + 2], f32)
        scratch = const.tile([128, SL], bf16)

        # dummy tiny sqrt: triggers the sqrt ACT_TABLE_LOAD during the
        # input DMA instead of lazily before the first real sqrt
        t_dummy = const.tile([128, 1], f32)
        nc.vector.memset(t_dummy[:], 1.0)
        dummyout = const.tile([128, 1], bf16)
        nc.scalar.activation(
            out=dummyout[:], in_=t_dummy[:], func=AF.Sqrt, scale=1.0
        )

        # single resident colsum accumulator [NCH, 512]
        cs_acc = cspool.tile([NCH, CT], f32, tag="cs", name="cs_acc")

        # zero the colsum accumulator (matmul with zero weights) and keep
        # the PE busy a few us so the clock gate opens before real work
        for rep in range(8):
            nc.tensor.matmul(
                cs_acc[:], t_zero10[:], t_z512[:],
                start=(rep == 0), stop=False, skip_group_check=True,
            )

        # ---- Sqrt phase: PE DR-65 fp8 matmuls -> PSUM, ACT sqrts ----
        ws = []
        last_sqrt = None
        for s in range(S):
            base = s * 128  # strip start in rotated cols
            w = wpool.tile([128, SL], bf16, tag="w")
            ws.append(w)
            lw = t_u2w[:, :, base:base + 128]
            if dve_strip[s]:
                # Schraudolph path: w holds s-codes' source A16*d
                scale = -(A16 * A16)
                bias = t_hsqpa[:, s:s + 1]
            else:
                scale = -1.0
                bias = t_hsqp[:, s:s + 1]
            # three 1024-col PSUM tiles (2 banks, double-buffered) plus a
            # final 1152-col tile (3 banks) absorbing the antipodal tail
            for t in range(4):
                c0 = t * PT
                if t < 3:
                    width = PT
                    ps = pspool.tile([128, PT], f32, tag="ps")
                    chunks = ((0, 512), (512, 1024))
                else:
                    width = 1152
                    ps = psbig.tile([128, 1152], f32, tag="psb")
                    chunks = ((0, 512), (512, 1024), (1024, 1152))
                for qa, qb in chunks:
                    nc.tensor.matmul(
                        ps[:, qa:qb],
                        lw,
                        t_u2[:, :, base + c0 + qa:base + c0 + qb],
                        start=True,
                        stop=not (t == 0 and qa == 0),
                        perf_mode=PM.DoubleRow,
                    )
                    if t == 0 and qa == 0:
                        # self block: nuke the diagonal (cols [0,128))
                        nc.tensor.matmul(
                            ps[:, 0:128],
                            t_ident[:],
                            t_dfix[:],
                            start=False,
                            stop=True,
                            skip_group_check=True,
                        )
                # w = sqrt(hsq_i - Q) (= d_ij, or A16*d_ij on DVE strips)
                last_sqrt = nc.scalar.activation(
                    out=w[:, c0:c0 + width],
                    in_=ps[:],
                    func=AF.Sqrt,
                    scale=scale,
                    bias=bias,
                )

        # ---- Exp phase + column sums ----
        # DVE strips run the Schraudolph pair as soon as their w exists;
        # ACT strips wait for the sqrt->exp table switch.
        def colsums(s, etile, rlo=128, rhi=CW):
            base = s * 128
            lo = base + rlo
            hi = base + rhi
            j = lo // CT
            while j * CT < hi:
                a = max(lo, j * CT)
                b = min(hi, (j + 1) * CT)
                nc.tensor.matmul(
                    cs_acc[:, a - j * CT:b - j * CT],
                    t_oneh[:, NCH * j:NCH * (j + 1)],
                    etile[:, a - base:b - base],
                    start=False,
                    stop=False,
                    skip_group_check=True,
                )
                j += 1

        for s in range(S):
            if not dve_strip[s]:
                continue
            w = ws[s]
            codes = cpool.tile([128, SL], i16, tag="codes")
            # codes = int16(B16 - w) = bf16 bits of ~exp(-d)
            # (w on DVE strips is already A16*d via the sqrt scale;
            #  plain tensor_tensor is the only DVE op with a 2x uop)
            nc.vector.tensor_tensor(
                out=codes[:], in0=t_b16[:], in1=w[:], op=OP.subtract,
            )
            # row sums of the decoded bf16 values: two 2x-mode folds then
            # a 1x accumulate pass over the remaining quarter strip
            cb = codes[:].bitcast(bf16)
            nc.vector.tensor_tensor(
                out=scratch[:, 0:SL // 2], in0=cb[:, 0:SL // 2],
                in1=cb[:, SL // 2:SL], op=OP.add,
            )
            nc.vector.tensor_tensor(
                out=scratch[:, SL // 2:SL // 2 + SL // 4],
                in0=scratch[:, 0:SL // 4],
                in1=scratch[:, SL // 4:SL // 2], op=OP.add,
            )
            nc.vector.tensor_scalar(
                out=scratch[:, 0:SL // 4],
                in0=scratch[:, SL // 2:SL // 2 + SL // 4],
                scalar1=1.0, scalar2=0.0,
                op0=OP.mult, op1=OP.add,
                accum_out=s0_t[:, s:s + 1],
            )
            colsums(s, codes[:].bitcast(bf16))

        for s in range(S):
            if dve_strip[s]:
                continue
            w = ws[s]
            dump = dpool.tile([128, SL], bf16, tag="dump")
            # three exp parts so colsums start as each part lands; the
            # extra accums go to spare s0 slots (host adds them back)
            parts = ((0, 1408, s), (1408, 2816, S), (2816, SL, S + 1))
            for pa, pb, slot in parts:
                e = nc.scalar.activation(
                    out=dump[:, pa:pb],
                    in_=w[:, pa:pb],
                    func=AF.Exp,
                    scale=-1.0,
                    bias=t_ebias[:],
                    accum_out=s0_t[:, slot:slot + 1],
                )
                if pa == 0 and last_sqrt is not None:
                    add_dep_helper(
                        e.ins, last_sqrt.ins, sync=False,
                        reason="ACT table phase: exp after all sqrts",
                    )
                colsums(s, dump[:], rlo=max(128, pa), rhi=min(CW, pb))

        # drain colsum accumulator: PSUM -> SBUF -> DRAM
        sb = misc.tile([NCH, CT], f32, tag="csdrain")
        nc.vector.tensor_copy(sb[:], cs_acc[:])
        nc.sync.dma_start(out=h_cs[:], in_=sb[:])
        nc.sync.dma_start(out=h_s0[:], in_=s0_t[:])

    nc.finalize()
    return nc


def get_program():
    if "nc" not in _cache:
        _cache["nc"] = _build_program()
    return _cache["nc"]


def make_in_maps(x, y):
    """Host-side prep: build the per-core (column-rotated) operand arrays."""
    from concourse import mybir

    FP8 = np.dtype(mybir.dt.np(mybir.dt.float8e4))

    x = np.asarray(x, dtype=np.float32)
    y = np.asarray(y, dtype=np.float32)
    z = np.concatenate([x, y], axis=0)  # [N, D]

    u8 = (np.float32(np.sqrt(2.0)) * z).astype(FP8)        # [N, D] fp8
    uf = u8.astype(np.float32)
    hsq = np.float32(0.5) * (uf * uf).sum(axis=1, dtype=np.float32)
    hsq_hi = hsq.astype(FP8)
    hsq_lo = (hsq - hsq_hi.astype(np.float32)).astype(FP8)

    ut = np.ascontiguousarray(uf.T)  # [D, N] f32 of the fp8 values

    dfix = np.zeros((128, 128), dtype=BF16)
    idx = np.arange(128)
    dfix[idx, idx] = BF16(-LARGE)
    ident = np.eye(128, dtype=BF16)
    oneh = np.zeros((128, NCH * NCH), dtype=BF16)
    for j in range(NCH):
        oneh[:, NCH * j + j] = BF16(1.0)

    in_maps = []
    for c in range(NCORES):
        r0 = c * RPC
        rows = np.arange(r0, r0 + RPC)

        def rotc(a):  # rotate columns of [*, N] by -r0, crop to UCOLS
            return np.roll(a, -r0, axis=-1)[..., :UCOLS]

        utr = rotc(ut)                       # [128, UCOLS] f32
        hhr = rotc(hsq_hi[None, :])[0]       # [UCOLS] fp8
        hlr = rotc(hsq_lo[None, :])[0]       # [UCOLS] fp8

        u2 = np.zeros((65, 2, UCOLS), dtype=FP8)
        u2[0:64, 0, :] = utr[0:64].astype(FP8)
        u2[0:64, 1, :] = utr[64:128].astype(FP8)
        u2[64, 0, :] = -hhr
        u2[64, 1, :] = -hlr

        u2w = np.zeros((65, 2, S * 128), dtype=FP8)
        u2w[0:64, 0, :] = utr[0:64, :S * 128].astype(FP8)
        u2w[0:64, 1, :] = utr[64:128, :S * 128].astype(FP8)
        u2w[64, 0, :] = np.float32(1.0).astype(FP8)
        u2w[64, 1, :] = np.float32(1.0).astype(FP8)

        def pcol(vec, sel):  # [RPC] values -> [128, S] per-partition layout
            return np.ascontiguousarray(vec[sel].reshape(S, 128).T)

        hp = pcol(hsq, rows)
        consts = np.concatenate(
            [
                dfix.view(np.uint8).reshape(128, -1),
                ident.view(np.uint8).reshape(128, -1),
                oneh.view(np.uint8).reshape(128, -1),
                hp.view(np.uint8).reshape(128, -1),
                (hp * np.float32(A16 * A16)).view(np.uint8).reshape(128, -1),
            ],
            axis=1,
        )
        in_maps.append(
            {
                "u2": u2,
                "u2w": u2w,
                "consts": np.ascontiguousarray(consts),
            }
        )
    return in_maps


def finish_on_host(results, x, y):
    """Gather per-core row sums + column sums; final loss with host dp."""
    S0 = np.zeros(N, dtype=np.float64)
    for c in range(NCORES):
        r0 = c * RPC
        s0 = np.asarray(results[c]["s0"], dtype=np.float64)  # [128, S+2]
        cs = np.asarray(results[c]["cs"], dtype=np.float64)  # [NCH, CT]
        # ACT strip's 2nd/3rd exp-part accums live in the spare slots
        s0[:, N_DVE] += s0[:, S] + s0[:, S + 1]
        S0[r0:r0 + RPC] += s0[:, :S].T.reshape(-1)
        # accumulated column sums: rotated col r in [128, 4992) holds the
        # core's total colsum for global row (r0 + r) mod N
        csf = cs.reshape(-1)
        rot = np.arange(128, (S - 1) * 128 + CW)
        gidx = (r0 + rot) % N
        S0[gidx] += csf[rot]

    z = np.concatenate([np.asarray(x, np.float64), np.asarray(y, np.float64)])
    dp = np.sqrt(((z[:B] - z[B:]) ** 2).sum(axis=1))
    DP = np.concatenate([dp, dp])

    S0 /= SCALE_COMP
    tiny = float(np.finfo(np.float32).tiny)
    num = np.exp(-DP)
    loss = -np.log(num / S0 + tiny)
    return np.asarray(loss.mean(), dtype=np.float32)


def kernel(x, y):
    global LAST_RESULT
    from concourse.bass_utils import run_bass_kernel_spmd

    nc = get_program()
    in_maps = make_in_maps(x, y)
    res = run_bass_kernel_spmd(
        nc, in_maps, list(range(NCORES)), trace=PROFILE
    )
    LAST_RESULT = res
    return finish_on_host(res.results, x, y)


# revision 58
# speedup vs baseline: 1.0267x; 1.0267x over previous
"""SNN (soft-nearest-neighbor) contrastive loss on 8 Trainium2 NeuronCores.

Math
----
z = concat(x, y) in R^{8192x128};  d_ij = ||z_i - z_j||.
The row max subtracted in the reference cancels mathematically, so
    S0_i  = sum_{j != i} exp(-d_ij)          (device + host gather)
    dp_i  = d_{i, pair(i)}                   (host, O(N*D))
    loss  = mean_i( -log( exp(-dp_i)/S0_i + tiny ) )   (host, trivial)

Symmetry halving
----------------
d_ij is symmetric; each 128-row block R computes exp tiles for column
blocks R..R+32 only (self + 32 forward, cyclically).  Strip = 4224 cols.
Row sums cover the WHOLE strip (the antipodal block, offset 32, is
computed by both partners for their own rows).  Column sums (PE one-hot
matmul into a single [10,512] PSUM accumulator) cover offsets 1..31 and
are scattered on the host into the mirrored rows.

Device pipeline (one SPMD program, 8 cores, rows sharded 1024/core)
------------------------------------------------------------------
PE: fp8e4 DoubleRow matmuls with 65-row k-tiles compute
      Q = u.u^T - hsq_j      (u split into 2x64 dims; the 65th row of
k-tile 0/1 carries ones x -hsq_hi / ones x -hsq_lo, folding the hsq_j
rank-1 term into the main matmul at ~fp16 precision for free, at
0.5 cyc/row), plus an FD-128 bf16 identity matmul adding -LARGE on
the self diagonal.
ACT: w = Sqrt(-Q + hsq_i) straight from PSUM via the per-partition
bias AP (bf16 out), one instruction per triple-buffered [128,1024]
PSUM tile.  Exp is split: the last strip runs on ACT (exp with fused
accum_out row sums; one sqrt->exp table switch, hoisted initial sqrt
table load); N_DVE strips run on DVE via the Schraudolph bit trick --
their sqrt is pre-scaled by A16^2 so a single 2x-mode tensor_tensor
subtract produces codes = int16(B16 - A16*d) = the bf16 bit pattern
of ~exp(-d)*SCALE_COMP.  Row sums of the bitcast-bf16 codes come from
two 2x-mode fold adds plus one accumulate pass; PE accumulates column
sums of every strip into one [10,512] PSUM tile via one-hot lhsT.
Each core gets column-ROTATED operands so every tile index is a
compile-time constant: one identical program for all 8 cores.
"""

import os
import sys
from contextlib import ExitStack

import numpy as np

_TRN_REPO = os.environ.get("TRN_RL_REPO", "/opt/trn_rl_repo")
if _TRN_REPO not in sys.path:
    sys.path.insert(0, _TRN_REPO)

import ml_dtypes

BF16 = ml_dtypes.bfloat16

B = 4096
D = 128
N = 2 * B            # 8192 rows of z
NCORES = 8
RPC = N // NCORES    # 1024 rows per core
S = RPC // 128       # 8 row-subtiles per core
CT = 512             # matmul moving tile (one PSUM bank = 512 f32)
SL = 4224            # strip length: self block + 32 forward blocks
CW = 4096            # colsum window end (blocks 1..31): [base+128, base+CW)
PT = 1024            # PSUM strip tile columns (2 banks)
UCOLS = 5120         # rotated cols touched: [0, 128*(S-1) + SL) = 5120
NCH = 10             # colsum chunks of 512 covering rotated cols [0, 5120)
LARGE = 7296.0       # diagonal nuke: d2 -> 7296, w -> 85.4:
                     #   ACT path exp(-85.4) ~ 8e-38 (bf16 ~ 0)
                     #   DVE path code = B16 - A16*85.4 ~ +474 (tiny value)
N_DVE = 7            # strips whose exp runs on DVE (Schraudolph)

LN2 = float(np.log(2.0))
A16 = 128.0 / LN2    # bf16 exponent-code slope
B16 = 16256.0        # bf16 exponent-code offset (exact in bf16)
# with B16=16256 the Schraudolph decode averages exp(-w)*SCALE_COMP;
# ACT-strip exps are biased by ln(SCALE_COMP) to match, and the host
# divides all device sums by SCALE_COMP.
SCALE_COMP = 1.0406027025852233  # mean of (1+f)/2^f over f~U[0,1)

PROFILE = False
LAST_RESULT = None

_cache = {}


def _build_program():
    import concourse.tile as tile
    from bass_rust import add_dep_helper
    from concourse import bacc, mybir

    f32 = mybir.dt.float32
    f16 = mybir.dt.float16
    bf16 = mybir.dt.bfloat16
    i16 = mybir.dt.int16
    fp8 = mybir.dt.float8e4
    AF = mybir.ActivationFunctionType
    OP = mybir.AluOpType
    PM = mybir.MatmulPerfMode

    nc = bacc.Bacc()

    u8 = mybir.dt.uint8
    h_u2 = nc.declare_dram_parameter("u2", [65, 2, UCOLS], fp8, isOutput=False)
    h_u2w = nc.declare_dram_parameter("u2w", [65, 2, S * 128], fp8, isOutput=False)
    # packed consts, one DMA: dfix | ident | oneh | hsqp | hsqpa (bytes)
    h_consts = nc.declare_dram_parameter("consts", [128, 776], u8, isOutput=False)
    h_s0 = nc.declare_dram_parameter("s0", [128, S + 2], f32, isOutput=True)
    h_cs = nc.declare_dram_parameter("cs", [NCH, CT], f32, isOutput=True)

    dve_strip = [s < N_DVE for s in range(S)]

    # strip for subtile s covers rotated cols [s*128, s*128 + SL)
    with tile.TileContext(nc) as tc, ExitStack() as ctx:
        const = ctx.enter_context(tc.tile_pool(name="const", bufs=1))
        wpool = ctx.enter_context(tc.tile_pool(name="wbuf", bufs=S))
        dpool = ctx.enter_context(tc.tile_pool(name="dump", bufs=2))
        cpool = ctx.enter_context(tc.tile_pool(name="codes", bufs=2))
        pspool = ctx.enter_context(tc.tile_pool(name="ps", bufs=3, space="PSUM"))
        pstail = ctx.enter_context(tc.tile_pool(name="pst", bufs=1, space="PSUM"))
        cspool = ctx.enter_context(tc.tile_pool(name="cps", bufs=1, space="PSUM"))
        misc = ctx.enter_context(tc.tile_pool(name="misc", bufs=1))

        # DMA priority: first matmul's operands first, then packed consts,
        # then the rest of u2 in chunks
        t_u2w = const.tile([65, 2, S * 128], fp8)
        nc.sync.dma_start(out=t_u2w[:], in_=h_u2w[:])
        t_u2 = const.tile([65, 2, UCOLS], fp8)
        edges = [0, 256, 512, 768, 1024, 1536, 2048, 2560, 3072, 3584,
                 4096, 4608, 5120]
        for a, b in zip(edges[:5], edges[1:6]):
            nc.sync.dma_start(out=t_u2[:, :, a:b], in_=h_u2[:, :, a:b])

        t_consts = const.tile([128, 776], u8)
        nc.sync.dma_start(out=t_consts[:], in_=h_consts[:])
        t_dfix = t_consts[:, 0:256].bitcast(bf16)
        t_ident = t_consts[:, 256:512].bitcast(bf16)
        t_oneh = t_consts[:, 512:712].bitcast(bf16)
        t_hsqp = t_consts[:, 712:744].bitcast(f32)
        t_hsqpa = t_consts[:, 744:776].bitcast(f32)

        for a, b in zip(edges[5:-1], edges[6:]):
            nc.sync.dma_start(out=t_u2[:, :, a:b], in_=h_u2[:, :, a:b])

        t_zero10 = const.tile([128, NCH], bf16)
        nc.vector.memset(t_zero10[:], 0.0)
        t_z512 = const.tile([128, CT], bf16)
        nc.vector.memset(t_z512[:], 0.0)
        t_b16 = const.tile([128, SL], bf16)
        nc.gpsimd.memset(t_b16[:], B16)
        t_ebias = const.tile([128, 1], f32)
        nc.vector.memset(t_ebias[:], float(np.log(SCALE_COMP)))

        s0_t = const.tile([128, S + 2], f32)
        scratch = const.tile([128, SL], bf16)

        # dummy tiny sqrt: triggers the sqrt ACT_TABLE_LOAD during the
        # input DMA instead of lazily before the first real sqrt
        t_dummy = const.tile([128, 1], f32)
        nc.vector.memset(t_dummy[:], 1.0)
        dummyout = const.tile([128, 1], bf16)
        nc.scalar.activation(
            out=dummyout[:], in_=t_dummy[:], func=AF.Sqrt, scale=1.0
        )

        # single resident colsum accumulator [NCH, 512]
        cs_acc = cspool.tile([NCH, CT], f32, tag="cs", name="cs_acc")

        # zero the colsum accumulator (matmul with zero weights) and keep
        # the PE busy a few us so the clock gate opens before real work
        for rep in range(8):
            nc.tensor.matmul(
                cs_acc[:], t_zero10[:], t_z512[:],
                start=(rep == 0), stop=False, skip_group_check=True,
            )

        # ---- Sqrt phase: PE DR-65 fp8 matmuls -> PSUM, ACT sqrts ----
        ws = []
        last_sqrt = None
        for s in range(S):
            base = s * 128  # strip start in rotated cols
            w = wpool.tile([128, SL], bf16, tag="w")
            ws.append(w)
            lw = t_u2w[:, :, base:base + 128]
            if dve_strip[s]:
                # Schraudolph path: w holds s-codes' source A16*d
                scale = -(A16 * A16)
                bias = t_hsqpa[:, s:s + 1]
            else:
                scale = -1.0
                bias = t_hsqp[:, s:s + 1]
            # four 1024-col PSUM tiles (2 banks each, triple-buffered)
            # plus a 128-col tail tile
            for t in range(4):
                c0 = t * PT
                ps = pspool.tile([128, PT], f32, tag="ps")
                for qa, qb in ((0, 512), (512, 1024)):
                    nc.tensor.matmul(
                        ps[:, qa:qb],
                        lw,
                        t_u2[:, :, base + c0 + qa:base + c0 + qb],
                        start=True,
                        stop=not (t == 0 and qa == 0),
                        perf_mode=PM.DoubleRow,
                    )
                    if t == 0 and qa == 0:
                        # self block: nuke the diagonal (cols [0,128))
                        nc.tensor.matmul(
                            ps[:, 0:128],
                            t_ident[:],
                            t_dfix[:],
                            start=False,
                            stop=True,
                            skip_group_check=True,
                        )
                # w = sqrt(hsq_i - Q) (= d_ij, or A16*d_ij on DVE strips)
                last_sqrt = nc.scalar.activation(
                    out=w[:, c0:c0 + PT],
                    in_=ps[:],
                    func=AF.Sqrt,
                    scale=scale,
                    bias=bias,
                )
            pst = pstail.tile([128, 128], f32, tag="pst")
            nc.tensor.matmul(
                pst[:],
                lw,
                t_u2[:, :, base + SL - 128:base + SL],
                start=True,
                stop=True,
                perf_mode=PM.DoubleRow,
            )
            last_sqrt = nc.scalar.activation(
                out=w[:, SL - 128:SL],
                in_=pst[:],
                func=AF.Sqrt,
                scale=scale,
                bias=bias,
            )

        # ---- Exp phase + column sums ----
        # DVE strips run the Schraudolph pair as soon as their w exists;
        # ACT strips wait for the sqrt->exp table switch.
        def colsums(s, etile, rlo=128, rhi=CW):
            base = s * 128
            lo = base + rlo
            hi = base + rhi
            j = lo // CT
            while j * CT < hi:
                a = max(lo, j * CT)
                b = min(hi, (j + 1) * CT)
                nc.tensor.matmul(
                    cs_acc[:, a - j * CT:b - j * CT],
                    t_oneh[:, NCH * j:NCH * (j + 1)],
                    etile[:, a - base:b - base],
                    start=False,
                    stop=False,
                    skip_group_check=True,
                )
                j += 1

        for s in range(S):
            if not dve_strip[s]:
                continue
            w = ws[s]
            codes = cpool.tile([128, SL], i16, tag="codes")
            # codes = int16(B16 - w) = bf16 bits of ~exp(-d)
            # (w on DVE strips is already A16*d via the sqrt scale;
            #  plain tensor_tensor is the only DVE op with a 2x uop)
            nc.vector.tensor_tensor(
                out=codes[:], in0=t_b16[:], in1=w[:], op=OP.subtract,
            )
            # row sums of the decoded bf16 values: two 2x-mode folds then
            # a 1x accumulate pass over the remaining quarter strip
            cb = codes[:].bitcast(bf16)
            nc.vector.tensor_tensor(
                out=scratch[:, 0:SL // 2], in0=cb[:, 0:SL // 2],
                in1=cb[:, SL // 2:SL], op=OP.add,
            )
            nc.vector.tensor_tensor(
                out=scratch[:, SL // 2:SL // 2 + SL // 4],
                in0=scratch[:, 0:SL // 4],
                in1=scratch[:, SL // 4:SL // 2], op=OP.add,
            )
            nc.vector.tensor_scalar(
                out=scratch[:, 0:SL // 4],
                in0=scratch[:, SL // 2:SL // 2 + SL // 4],
                scalar1=1.0, scalar2=0.0,
                op0=OP.mult, op1=OP.add,
                accum_out=s0_t[:, s:s + 1],
            )
            colsums(s, codes[:].bitcast(bf16))

        for s in range(S):
            if dve_strip[s]:
                continue
            w = ws[s]
            dump = dpool.tile([128, SL], bf16, tag="dump")
            # three exp parts so colsums start as each part lands; the
            # extra accums go to spare s0 slots (host adds them back)
            parts = ((0, 1408, s), (1408, 2816, S), (2816, SL, S + 1))
            for pa, pb, slot in parts:
                e = nc.scalar.activation(
                    out=dump[:, pa:pb],
                    in_=w[:, pa:pb],
                    func=AF.Exp,
                    scale=-1.0,
                    bias=t_ebias[:],
                    accum_out=s0_t[:, slot:slot + 1],
                )
                if pa == 0 and last_sqrt is not None:
                    add_dep_helper(
                        e.ins, last_sqrt.ins, sync=False,
                        reason="ACT table phase: exp after all sqrts",
                    )
                colsums(s, dump[:], rlo=max(128, pa), rhi=min(CW, pb))

        # drain colsum accumulator: PSUM -> SBUF -> DRAM
        sb = misc.tile([NCH, CT], f32, tag="csdrain")
        nc.vector.tensor_copy(sb[:], cs_acc[:])
        nc.sync.dma_start(out=h_cs[:], in_=sb[:])
        nc.sync.dma_start(out=h_s0[:], in_=s0_t[:])

    nc.finalize()
    return nc


def get_program():
    if "nc" not in _cache:
        _cache["nc"] = _build_program()
    return _cache["nc"]


def make_in_maps(x, y):
    """Host-side prep: build the per-core (column-rotated) operand arrays."""
    from concourse import mybir

    FP8 = np.dtype(mybir.dt.np(mybir.dt.float8e4))

    x = np.asarray(x, dtype=np.float32)
    y = np.asarray(y, dtype=np.float32)
    z = np.concatenate([x, y], axis=0)  # [N, D]

    u8 = (np.float32(np.sqrt(2.0)) * z).astype(FP8)        # [N, D] fp8
    uf = u8.astype(np.float32)
    hsq = np.float32(0.5) * (uf * uf).sum(axis=1, dtype=np.float32)
    hsq_hi = hsq.astype(FP8)
    hsq_lo = (hsq - hsq_hi.astype(np.float32)).astype(FP8)

    ut = np.ascontiguousarray(uf.T)  # [D, N] f32 of the fp8 values

    dfix = np.zeros((128, 128), dtype=BF16)
    idx = np.arange(128)
    dfix[idx, idx] = BF16(-LARGE)
    ident = np.eye(128, dtype=BF16)
    oneh = np.zeros((128, NCH * NCH), dtype=BF16)
    for j in range(NCH):
        oneh[:, NCH * j + j] = BF16(1.0)

    in_maps = []
    for c in range(NCORES):
        r0 = c * RPC
        rows = np.arange(r0, r0 + RPC)

        def rotc(a):  # rotate columns of [*, N] by -r0, crop to UCOLS
            return np.roll(a, -r0, axis=-1)[..., :UCOLS]

        utr = rotc(ut)                       # [128, UCOLS] f32
        hhr = rotc(hsq_hi[None, :])[0]       # [UCOLS] fp8
        hlr = rotc(hsq_lo[None, :])[0]       # [UCOLS] fp8

        u2 = np.zeros((65, 2, UCOLS), dtype=FP8)
        u2[0:64, 0, :] = utr[0:64].astype(FP8)
        u2[0:64, 1, :] = utr[64:128].astype(FP8)
        u2[64, 0, :] = -hhr
        u2[64, 1, :] = -hlr

        u2w = np.zeros((65, 2, S * 128), dtype=FP8)
        u2w[0:64, 0, :] = utr[0:64, :S * 128].astype(FP8)
        u2w[0:64, 1, :] = utr[64:128, :S * 128].astype(FP8)
        u2w[64, 0, :] = np.float32(1.0).astype(FP8)
        u2w[64, 1, :] = np.float32(1.0).astype(FP8)

        def pcol(vec, sel):  # [RPC] values -> [128, S] per-partition layout
            return np.ascontiguousarray(vec[sel].reshape(S, 128).T)

        hp = pcol(hsq, rows)
        consts = np.concatenate(
            [
                dfix.view(np.uint8).reshape(128, -1),
                ident.view(np.uint8).reshape(128, -1),
                oneh.view(np.uint8).reshape(128, -1),
                hp.view(np.uint8).reshape(128, -1),
                (hp * np.float32(A16 * A16)).view(np.uint8).reshape(128, -1),
            ],
            axis=1,
        )
        in_maps.append(
            {
                "u2": u2,
                "u2w": u2w,
                "consts": np.ascontiguousarray(consts),
            }
        )
    return in_maps


def finish_on_host(results, x, y):
    """Gather per-core row sums + column sums; final loss with host dp."""
    S0 = np.zeros(N, dtype=np.float64)
    for c in range(NCORES):
        r0 = c * RPC
        s0 = np.asarray(results[c]["s0"], dtype=np.float64)  # [128, S+2]
        cs = np.asarray(results[c]["cs"], dtype=np.float64)  # [NCH, CT]
        # ACT strip's 2nd/3rd exp-part accums live in the spare slots
        s0[:, N_DVE] += s0[:, S] + s0[:, S + 1]
        S0[r0:r0 + RPC] += s0[:, :S].T.reshape(-1)
        # accumulated column sums: rotated col r in [128, 4992) holds the
        # core's total colsum for global row (r0 + r) mod N
        csf = cs.reshape(-1)
        rot = np.arange(128, (S - 1) * 128 + CW)
        gidx = (r0 + rot) % N
        S0[gidx] += csf[rot]

    z = np.concatenate([np.asarray(x, np.float64), np.asarray(y, np.float64)])
    dp = np.sqrt(((z[:B] - z[B:]) ** 2).sum(axis=1))
    DP = np.concatenate([dp, dp])

    S0 /= SCALE_COMP
    tiny = float(np.finfo(np.float32).tiny)
    num = np.exp(-DP)
    loss = -np.log(num / S0 + tiny)
    return np.asarray(loss.mean(), dtype=np.float32)


def kernel(x, y):
    global LAST_RESULT
    from concourse.bass_utils import run_bass_kernel_spmd

    nc = get_program()
    in_maps = make_in_maps(x, y)
    res = run_bass_kernel_spmd(
        nc, in_maps, list(range(NCORES)), trace=PROFILE
    )
    LAST_RESULT = res
    return finish_on_host(res.results, x, y)
